# revision 1
# baseline (speedup 1.0000x reference)
"""CenterLoss on 8 Trainium2 NeuronCores. Known-good v2: 14614 ns.

loss = sum_i ||x_i - centers[labels[i]]||^2 / (B * C), batch-sharded
across 8 cores; per-core row sums are combined on the host.
"""

import numpy as np

import concourse.bacc as bacc
import concourse.bass as bass
import concourse.mybir as mybir
from concourse.bass_utils import run_bass_kernel_spmd

B = 1024
C = 100000
D = 128
NCORES = 8
BS = B // NCORES

F32 = mybir.dt.float32
I32 = mybir.dt.int32

STRIP_CONST_MEMSETS = True

_NC_CACHE = {}


def _strip_const_memsets(nc):
    for f in nc.m.functions:
        for blk in f.blocks:
            blk.instructions = [
                i
                for i in blk.instructions
                if not (
                    type(i).__name__ == "InstMemset"
                    and i.outs
                    and "const-" in str(i.outs[0])
                )
            ]


def _build_nc():
    nc = bacc.Bacc("TRN2")

    x = nc.dram_tensor("x", [BS, D], F32, kind="ExternalInput")
    labels = nc.dram_tensor("labels", [BS, 1], I32, kind="ExternalInput")
    centers = nc.dram_tensor("centers", [C, D], F32, kind="ExternalInput")
    out = nc.dram_tensor("out", [BS, 1], F32, kind="ExternalOutput")

    with (
        nc.sbuf_tensor("x_t", [BS, D], F32) as x_t,
        nc.sbuf_tensor("lab_sb", [BS, 1], I32) as lab_sb,
        nc.sbuf_tensor("c_t", [BS, D], F32) as c_t,
        nc.sbuf_tensor("d_t", [BS, D], F32) as d_t,
        nc.sbuf_tensor("rowsum", [BS, 1], F32) as rowsum,
    ):
        d_lab = nc.alloc_semaphore("d_lab")
        d_x = nc.alloc_semaphore("d_x")
        d_g = nc.alloc_semaphore("d_g")
        d_out = nc.alloc_semaphore("d_out")
        s_dve = nc.alloc_semaphore("s_dve")

        nc.sync.dma_start(lab_sb.ap(), labels[:, :]).then_inc(d_lab, 16)
        nc.sync.dma_start(x_t.ap(), x[:, :]).then_inc(d_x, 16)
        nc.sync.wait_ge(s_dve, 3)
        nc.sync.dma_start(out[:, :], rowsum.ap()).then_inc(d_out, 16)

        nc.gpsimd.wait_ge(d_lab, 16)
        nc.gpsimd.indirect_dma_start(
            out=c_t.ap(),
            out_offset=None,
            in_=centers[:, :],
            in_offset=bass.IndirectOffsetOnAxis(ap=lab_sb.ap()[:, :1], axis=0),
        ).then_inc(d_g, 16)

        nc.vector.wait_ge(d_x, 16)
        nc.vector.wait_ge(d_g, 16)
        nc.vector.tensor_sub(d_t.ap(), x_t.ap(), c_t.ap()).then_inc(s_dve, 1)
        nc.vector.wait_ge(s_dve, 1)
        nc.vector.tensor_mul(d_t.ap(), d_t.ap(), d_t.ap()).then_inc(s_dve, 1)
        nc.vector.wait_ge(s_dve, 2)
        nc.vector.reduce_sum(
            rowsum.ap(), d_t.ap(), axis=mybir.AxisListType.X
        ).then_inc(s_dve, 1)

    if STRIP_CONST_MEMSETS:
        _strip_const_memsets(nc)
    nc.compile()
    return nc


def _run(x, labels, centers, **spmd_kwargs):
    x = np.ascontiguousarray(np.asarray(x, dtype=np.float32))
    centers = np.ascontiguousarray(np.asarray(centers, dtype=np.float32))
    labels_i32 = np.asarray(labels).astype(np.int32).reshape(NCORES, BS, 1)

    if "nc" not in _NC_CACHE:
        _NC_CACHE["nc"] = _build_nc()
    nc = _NC_CACHE["nc"]

    in_maps = [
        {
            "x": x[i * BS : (i + 1) * BS],
            "labels": np.ascontiguousarray(labels_i32[i]),
            "centers": centers,
        }
        for i in range(NCORES)
    ]
    res = run_bass_kernel_spmd(nc, in_maps, core_ids=list(range(NCORES)), **spmd_kwargs)

    total = float(
        np.sum([r["out"].astype(np.float64) for r in res.results], dtype=np.float64)
    )
    return np.float32(total / (B * C)), res


def kernel(x, labels, centers):
    loss, _ = _run(x, labels, centers)
    return loss



# revision 2
# speedup vs baseline: 2.4739x; 2.4739x over previous
"""CenterLoss v5: gather + DVE sub + fused ttr + PE ones-matmul -> scalar out.

The [1,1] output avoids the [128,1] SBUF->DRAM DMA whose per-partition 4B
reads trickle completion over ~5us and delay the NEFF teardown.
"""

import numpy as np

import concourse.bacc as bacc
import concourse.bass as bass
import concourse.mybir as mybir
from concourse.bass_utils import run_bass_kernel_spmd

B = 1024
C = 100000
D = 128
NCORES = 8
BS = B // NCORES

F32 = mybir.dt.float32
I32 = mybir.dt.int32

_NC_CACHE = {}


def _strip_const_memsets(nc):
    for f in nc.m.functions:
        for blk in f.blocks:
            blk.instructions = [
                i
                for i in blk.instructions
                if not (
                    type(i).__name__ == "InstMemset"
                    and i.outs
                    and "const-" in str(i.outs[0])
                )
            ]


def _build_nc():
    nc = bacc.Bacc("TRN2")

    x = nc.dram_tensor("x", [BS, D], F32, kind="ExternalInput")
    labels = nc.dram_tensor("labels", [BS, 1], I32, kind="ExternalInput")
    centers = nc.dram_tensor("centers", [C, D], F32, kind="ExternalInput")
    ones = nc.dram_tensor("ones", [BS, 1], F32, kind="ExternalInput")
    out = nc.dram_tensor("out", [1, 1], F32, kind="ExternalOutput")

    with (
        nc.sbuf_tensor("x_t", [BS, D], F32) as x_t,
        nc.sbuf_tensor("lab_sb", [BS, 1], I32) as lab_sb,
        nc.sbuf_tensor("c_t", [BS, D], F32) as c_t,
        nc.sbuf_tensor("d_t", [BS, D], F32) as d_t,
        nc.sbuf_tensor("sq_t", [BS, D], F32) as sq_t,
        nc.sbuf_tensor("rowsum", [BS, 1], F32) as rowsum,
        nc.sbuf_tensor("ones_t", [BS, 1], F32) as ones_t,
        nc.sbuf_tensor("res_t", [1, 1], F32) as res_t,
        nc.psum_tensor("ps", [1, 1], F32) as ps,
    ):
        d_lab = nc.alloc_semaphore("d_lab")
        d_x = nc.alloc_semaphore("d_x")
        d_g = nc.alloc_semaphore("d_g")
        d_out = nc.alloc_semaphore("d_out")
        s_v = nc.alloc_semaphore("s_v")
        s_mm = nc.alloc_semaphore("s_mm")

        nc.sync.dma_start(lab_sb.ap(), labels[:, :]).then_inc(d_lab, 16)
        nc.sync.dma_start(x_t.ap(), x[:, :]).then_inc(d_x, 16)
        nc.sync.dma_start(ones_t.ap(), ones[:, :]).then_inc(d_x, 16)

        nc.gpsimd.wait_ge(d_lab, 16)
        nc.gpsimd.indirect_dma_start(
            out=c_t.ap(),
            out_offset=None,
            in_=centers[:, :],
            in_offset=bass.IndirectOffsetOnAxis(ap=lab_sb.ap()[:, :1], axis=0),
        ).then_inc(d_g, 16)

        nc.vector.wait_ge(d_x, 32)
        nc.vector.wait_ge(d_g, 16)
        nc.vector.tensor_sub(d_t.ap(), x_t.ap(), c_t.ap())
        nc.vector.tensor_mul(sq_t.ap(), d_t.ap(), d_t.ap())
        nc.vector.reduce_sum(
            rowsum.ap(), sq_t.ap(), axis=mybir.AxisListType.X
        ).then_inc(s_v, 1)

        nc.tensor.wait_ge(s_v, 1)
        nc.tensor.matmul(ps.ap(), ones_t.ap(), rowsum.ap()).then_inc(s_mm, 1)

        nc.vector.wait_ge(s_mm, 1)
        nc.vector.tensor_copy(res_t.ap(), ps.ap()).then_inc(s_v, 1)

        nc.sync.wait_ge(s_v, 2)
        nc.sync.dma_start(out[:, :], res_t.ap()).then_inc(d_out, 16)

    _strip_const_memsets(nc)
    nc.compile()
    return nc


def _run(x, labels, centers, **spmd_kwargs):
    x = np.ascontiguousarray(np.asarray(x, dtype=np.float32))
    centers = np.ascontiguousarray(np.asarray(centers, dtype=np.float32))
    labels_i32 = np.asarray(labels).astype(np.int32).reshape(NCORES, BS, 1)
    ones = np.ones((BS, 1), dtype=np.float32)

    if "nc" not in _NC_CACHE:
        _NC_CACHE["nc"] = _build_nc()
    nc = _NC_CACHE["nc"]

    in_maps = [
        {
            "x": x[i * BS : (i + 1) * BS],
            "labels": np.ascontiguousarray(labels_i32[i]),
            "centers": centers,
            "ones": ones,
        }
        for i in range(NCORES)
    ]
    res = run_bass_kernel_spmd(nc, in_maps, core_ids=list(range(NCORES)), **spmd_kwargs)

    total = float(
        np.sum([r["out"].astype(np.float64) for r in res.results], dtype=np.float64)
    )
    return np.float32(total / (B * C)), res


def kernel(x, labels, centers):
    loss, _ = _run(x, labels, centers)
    return loss
